# revision 43
# baseline (speedup 1.0000x reference)
"""Trainium2 Bass kernel for batched single-head attention with seq-sum pooling.

Reference computation (B=16, S=2048, D=512, fp32):
    q = x @ W_q ; k = x @ W_k ; v = x @ W_v          per batch  [S, D]
    scores = q @ k.T / sqrt(D)                        [S, S]
    attn = softmax(scores, axis=-1)
    out_b = sum_s (attn @ v)[s, :]                    [D]

Algebraic restructures:
1. The final sum over query positions commutes through both trailing
   matmuls: out_b = ((r^T E) @ x) @ W_v with E = exp(scores/sqrt(D)) and
   r[q] = 1/rowsum_q(E) — removes the [S,S]x[S,D] attention-value matmul
   AND the V projection.
2. scores = x M x^T with M = W_q W_k^T computed once per core — one
   G = x M projection replaces both per-batch Q/K projections.

fp8: the G projection, scores, and softmax column-sum matmuls run with
float8e4 operands in DoubleRow mode (K=256/instruction).  Exact foldings
keep fp8 in range: M stored as 16*M, E = exp(s/sqrt(D) - 2.5) (global
offset cancels through the softmax), r stored as 128*r (folded into the
final output copy).  Measured rel err 7.5e-3 (tolerance 2e-2).

Scores/colsum stationaries are stored PRE-INTERLEAVED for
DoubleRowSwInterleave: the HW weight load then reads contiguously instead
of DoubleRow's strided interleave (which disables fast-weight-load and
makes LDWEIGHTS the ~260ns/matmul bottleneck).  The interleave's column
reversal permutes scores rows (q) within each 128-block — harmless, since
every consumer (exp, row-sum, r broadcast, weighted column-sum) is
q-order-free, and all per-q tensors carry the same permutation.  The G
matmul keeps plain DoubleRow (a reversed G would misalign the scores
contraction).

Schedule (from trace analysis: the PE re-throttles 2.4->1.2GHz on idle
windows, so density is everything): each batch's score rows are computed in
two half-row passes — phase A covers key chunks {0,1}, phase B {2,3} —
which lets phase A start after only 4 transposes + 4 G chunks (~6us) and
hides the entire 8MB x DMA under compute.  Row sums accumulate per-half
via exp's accum_out into persistent per-q-tile tiles; r and the colsums
happen in phase B.  Colsum kc{0,1} accumulates inline (2-pair lag);
kc{2,3} runs as a deferred sweep.  Weave plan: batch0's remaining
transposes/G chunks fill phase A0's slack; batch1's projection fills B0;
batch0's sweep+w-phase fills A1; the only serial tails are ~8us of fill
and batch1's w-phase.  PSUM: sc 2x[P,2,512] + gp 2x1 + w 2 banks = 8.

Sharding: pure data parallelism over batch — 2 batch elements per core on
8 NeuronCores, weights replicated, no collectives.
"""

import sys

sys.path.insert(0, "/opt/trn_rl_repo")

import numpy as np

import concourse.bass as bass
import concourse.mybir as mybir
import concourse.tile as tile
from concourse import bacc
from concourse.bass_utils import run_bass_kernel_spmd
from concourse.masks import make_identity

B, S, D = 16, 2048, 512
P = 128
N_CORES = 8
B_PER_CORE = B // N_CORES  # 2
SCALE = 1.0 / float(np.sqrt(D))
KM = 16.0  # M pre-scale (exact power of 2)
KR = 128.0  # r pre-scale (exact power of 2)
C_OFF = 2.5  # global exp offset; cancels through softmax normalization

F32 = mybir.dt.float32
BF16 = mybir.dt.bfloat16
F8 = mybir.dt.float8e4
DR = mybir.MatmulPerfMode.DoubleRow
SWI = mybir.MatmulPerfMode.DoubleRowSwInterleave
USE_SWI = True  # pre-interleaved scores/colsum stationaries

N_ST = S // P  # 16 s-tiles
N_DT = D // P  # 4 d-tiles
NCH = 512  # moving free dim per matmul (one fp32 PSUM bank)
N_SC = S // NCH  # 4 s-chunks
N_KC = S // NCH  # 4 k-chunks
N_PAIR = N_ST // 2  # 8 q-tile pairs


def build_nc():
    nc = bacc.Bacc("TRN2", target_bir_lowering=False, debug=False, num_devices=N_CORES)
    x_ext = nc.dram_tensor(
        "inputs", [B_PER_CORE, S, D], F32, kind="ExternalInput"
    ).ap()
    wq_ext = nc.dram_tensor("W_q", [D, D], F32, kind="ExternalInput").ap()
    wk_ext = nc.dram_tensor("W_k", [D, D], F32, kind="ExternalInput").ap()
    wv_ext = nc.dram_tensor("W_v", [D, D], F32, kind="ExternalInput").ap()
    out_ext = nc.dram_tensor("out", [B_PER_CORE, D], F32, kind="ExternalOutput").ap()

    with tile.TileContext(nc) as tc:
        with (
            tc.tile_pool(name="const", bufs=1) as const_pool,
            tc.tile_pool(name="w", bufs=1) as w_pool,
            tc.tile_pool(name="xnat", bufs=2) as xnat_pool,
            tc.tile_pool(name="xt", bufs=2) as xt_pool,
            tc.tile_pool(name="qkv", bufs=2) as qkv_pool,
            tc.tile_pool(name="e", bufs=17) as e_pool,
            tc.tile_pool(name="soft", bufs=4) as soft_pool,
            tc.tile_pool(name="rs", bufs=36) as rs_pool,
            tc.tile_pool(name="r2", bufs=18) as r2_pool,
            tc.tile_pool(name="wvec", bufs=2) as wvec_pool,
            tc.tile_pool(name="scps", bufs=2, space="PSUM") as sc_psum,
            tc.tile_pool(name="gpps", bufs=2, space="PSUM") as gp_psum,
            tc.tile_pool(name="wps", bufs=1, space="PSUM") as w_psum,
        ):
            one_t = const_pool.tile([1, 1], BF16)
            nc.gpsimd.memset(one_t[:], 1.0)
            ident_f = const_pool.tile([P, P], F32)
            make_identity(nc, ident_f[:])
            ident = const_pool.tile([P, P], BF16)
            nc.vector.tensor_copy(ident[:], ident_f[:])
            negc_t = const_pool.tile([P, 1], F32)
            nc.gpsimd.memset(negc_t[:], -C_OFF)

            # HAM warmup: the PE clock boots throttled to 1.2GHz and only
            # un-throttles to 2.4GHz after ~3.4us of sustained matmul
            # activity (one full busy window of the hardware activity
            # monitor) — and a DoubleRow/fp8 stream SUSTAINS the warm state
            # but was never observed to CREATE it.  Dependency-free bf16
            # dummy matmuls are woven BETWEEN the fill-phase units (whose
            # pace is copy-bound, leaving PE gaps) so the PE stream is
            # gapless from t~1us without delaying real work.
            warm_mov = const_pool.tile([P, NCH], BF16)
            nc.gpsimd.memset(warm_mov[:], 0.0)
            warm_ps = sc_psum.tile([P, 2, NCH], F32, tag="sc")
            warm_i = [0]

            def emit_warm_dummy(n=1):
                for _ in range(n):
                    nc.tensor.matmul(
                        warm_ps[:, warm_i[0] % 2, :],
                        ident[:],
                        warm_mov[:],
                        start=True,
                        stop=True,
                        skip_group_check=True,
                    )
                    warm_i[0] += 1

            # Solid block: the fill units are DMA-gated until ~6us anyway,
            # so a 10-matmul block (~4.5us cold) costs nothing and fires the
            # activity monitor just as real work arrives.
            emit_warm_dummy(10)

            def dma_x_chunk(b, sc, xnat_s):
                nc.gpsimd.dma_start(
                    out=xnat_s[:, sc * 4 : (sc + 1) * 4, :],
                    in_=x_ext[b, sc * NCH : (sc + 1) * NCH, :].rearrange(
                        "(t p) d -> p t d", p=P
                    ),
                )

            w_tiles = {}

            def dma_w(name, ext):
                w_s = w_pool.tile([P, N_DT, D], BF16, tag=name)
                nc.gpsimd.dma_start(
                    out=w_s[:], in_=ext.rearrange("(t p) e -> p t e", p=P)
                )
                w_tiles[name] = w_s

            # DMA plan.  One SWDGE dma_start of ~1MB costs ~6us and the queue
            # is FIFO, so the x chunks monopolize it: s1-3, then batch0
            # chunks 1-3, then ALL of batch1's chunks, then wv — each landing
            # just ahead of its consumer phase.  The weights ride the two
            # parallel HWDGE queues as plain f32 (HWDGE can't cast; the
            # prework transposes consume f32 directly), so M prework starts
            # at ~4us instead of ~12.  s-tile 0 is f32 on the sync queue.
            xnat0_s = xnat_pool.tile([P, N_ST, D], BF16, tag="xnat")
            xnat1_s = xnat_pool.tile([P, N_ST, D], BF16, tag="xnat")
            xf0 = xnat_pool.tile([P, D], F32, tag="xf0")
            wkf_s = w_pool.tile([P, N_DT, D], F32, tag="wkf")
            wqf_s = w_pool.tile([P, N_DT, D], F32, tag="wqf")
            nc.scalar.dma_start(
                out=wkf_s[:], in_=wk_ext.rearrange("(t p) e -> p t e", p=P)
            )
            nc.sync.dma_start(
                out=wqf_s[:], in_=wq_ext.rearrange("(t p) e -> p t e", p=P)
            )
            nc.sync.dma_start(out=xf0[:], in_=x_ext[0, 0:P, :])
            nc.vector.tensor_copy(xnat0_s[:, 0, :], xf0[:])
            nc.gpsimd.dma_start(
                out=xnat0_s[:, 1:4, :],
                in_=x_ext[0, P:NCH, :].rearrange("(t p) d -> p t d", p=P),
            )
            dma_x_chunk(0, 1, xnat0_s)
            dma_x_chunk(0, 2, xnat0_s)
            dma_x_chunk(0, 3, xnat0_s)
            for sc in range(N_SC):
                dma_x_chunk(1, sc, xnat1_s)
            dma_w("wv", wv_ext)
            x0_loaded = [True] * N_SC
            wv_s = w_tiles["wv"]

            # One-time prework: M = Wq Wk^T, stored as 16*M fp8 (raw entries
            # would be fp8-subnormal; the exp scale divides the 16 out).
            # wqT scaled by 16 at its ACT copy; wkT copies also on ACT so the
            # fill phase's DVE stays on the x transposes.
            wqT_s = w_pool.tile([P, N_DT, D], BF16, tag="wqT")
            wkT_s = w_pool.tile([P, N_DT, D], BF16, tag="wkT")
            m8_s = w_pool.tile([P, N_DT, D], F8, tag="m8")

            # During the fill the sc and w PSUM pools are idle; cycling the
            # fill units across all three pools gives 4-5 concurrent
            # unit-copy lanes instead of serializing ~20 copies through gp's
            # two banks.
            # sc stays out of the fill cycle: its tag ring is shared with
            # warm_ps, and a fill unit landing on that slot inherits a
            # write-after-write chain on the dummy matmuls.
            fill_pools = [gp_psum, w_psum, gp_psum]
            fill_pi = [0]

            def next_fill_pool():
                p = fill_pools[fill_pi[0] % len(fill_pools)]
                fill_pi[0] += 1
                return p

            a0_pools = [gp_psum, w_psum]
            a0_pi = [0]

            def pool_for(ctx):
                # "fill": sc/w pools are idle -> 4 lanes.  "a0": scores own
                # sc, colsum hasn't started -> gp + w.  "b0": only gp free.
                if ctx == "fill":
                    return next_fill_pool()
                if ctx == "a0":
                    p = a0_pools[a0_pi[0] % 2]
                    a0_pi[0] += 1
                    return p
                return gp_psum

            def pool_tag(pool):
                # reuse each pool's canonical tag: a new tag would get its
                # own buffer ring and blow the 8-bank PSUM budget
                if pool is sc_psum:
                    return "sc"
                if pool is w_psum:
                    return "w"
                return "gp"

            def m_prework_thunks():
                thunks = []

                def make_wtrans_unit(src_w, dst, t_e, scale):
                    def th():
                        pool = next_fill_pool()
                        tp = pool.tile([P, N_DT * P], F32, tag=pool_tag(pool))
                        for t_a in range(N_DT):
                            nc.tensor.matmul(
                                tp[:, t_a * P : (t_a + 1) * P],
                                src_w[:, t_a, t_e * P : (t_e + 1) * P],
                                ident_f[:],
                                start=True,
                                stop=True,
                                skip_group_check=True,
                            )
                        # wkT on ACT, wqT (scaled) on DVE: two parallel
                        # copy chains for the prework
                        if scale is None:
                            nc.scalar.copy(dst[:, t_e, :], tp[:])
                        else:
                            nc.vector.tensor_scalar_mul(dst[:, t_e, :], tp[:], scale)

                    return th

                def make_m_group(t_a):
                    def th():
                        pool = next_fill_pool()
                        mp = pool.tile([P, NCH], F32, tag=pool_tag(pool))
                        for t_e in range(N_DT):
                            nc.tensor.matmul(
                                mp[:],
                                wqT_s[:, t_e, t_a * P : (t_a + 1) * P],
                                wkT_s[:, t_e, :],
                                start=(t_e == 0),
                                stop=(t_e == N_DT - 1),
                            )
                        nc.vector.tensor_copy(m8_s[:, t_a, :], mp[:])

                    return th

                for t_e in range(N_DT):
                    thunks.append(make_wtrans_unit(wkf_s, wkT_s, t_e, None))
                for t_e in range(N_DT):
                    thunks.append(make_wtrans_unit(wqf_s, wqT_s, t_e, KM))
                for t_a in range(N_DT):
                    thunks.append(make_m_group(t_a))
                return thunks

            # ---------- thunk builders --------------------------------------

            def proj_thunks(b, xnat_s, loaded, unit_ctx=None):
                """Transpose + G = X M projection thunks for batch b.  xt8 is
                [P, dtile, S] fp8.  gt8 layout depends on USE_SWI:
                  - SWI: [P, jpair, qt_block, 2*P] with the two d-subtiles of
                    a jpair interleaved along the last dim (stored UNreversed;
                    the HW's column reversal permutes q within blocks, which
                    every downstream consumer absorbs).
                  - plain DR: [P, dtile, S]."""
                xt8_s = xt_pool.tile([P, N_DT, S], F8, tag="xt")
                if USE_SWI:
                    gt8_s = qkv_pool.tile([P, 2, N_ST, 2 * P], F8, tag="gt")
                else:
                    gt8_s = qkv_pool.tile([P, N_DT, S], F8, tag="gt")

                def make_dma(sc):
                    def th():
                        dma_x_chunk(b, sc, xnat_s)

                    return th

                dma_th = [
                    None if loaded[sc] else make_dma(sc) for sc in range(N_SC)
                ]

                def make_trans_unit(sc, t_i):
                    def th():
                        st = sc * 4 + t_i
                        pool = pool_for(unit_ctx[sc] if unit_ctx else "b0")
                        tp = pool.tile([P, N_DT * P], F32, tag=pool_tag(pool))
                        for dt_i in range(N_DT):
                            nc.tensor.matmul(
                                tp[:, dt_i * P : (dt_i + 1) * P],
                                xnat_s[:, st, dt_i * P : (dt_i + 1) * P],
                                ident[:],
                                start=True,
                                stop=True,
                                skip_group_check=True,
                            )
                        nc.vector.tensor_copy(
                            xt8_s[:, :, st * P : (st + 1) * P],
                            tp[:].rearrange("p (t c) -> p t c", t=N_DT),
                        )

                    return th

                trans_th = [
                    [make_trans_unit(sc, t_i) for t_i in range(4)]
                    for sc in range(N_SC)
                ]

                def make_g(sc, ct):
                    def th():
                        ctx = unit_ctx[sc] if unit_ctx else "b0"
                        pool = pool_for(ctx)
                        mp = pool.tile([P, NCH], F32, tag=pool_tag(pool))
                        for j in range(2):
                            nc.tensor.matmul(
                                mp[:],
                                m8_s[:, 2 * j : 2 * j + 2, ct * P : (ct + 1) * P],
                                xt8_s[:, 2 * j : 2 * j + 2, sc * NCH : (sc + 1) * NCH],
                                start=(j == 0),
                                stop=(j == 1),
                                perf_mode=DR,
                            )
                        # in the fill, ACT is idle: split the casts across
                        # both PSUM-capable copy engines
                        eng = nc.scalar if (ctx == "fill" and ct % 2 == 0) else None
                        if USE_SWI:
                            (eng.copy if eng else nc.vector.tensor_copy)(
                                gt8_s[
                                    :,
                                    ct // 2,
                                    sc * 4 : (sc + 1) * 4,
                                    (ct % 2) :: 2,
                                ],
                                mp[:].rearrange("p (b q) -> p b q", b=4),
                            )
                        else:
                            nc.vector.tensor_copy(
                                gt8_s[:, ct, sc * NCH : (sc + 1) * NCH], mp[:]
                            )

                    return th

                kq_th = [
                    [make_g(sc, ct) for ct in range(N_DT)]
                    for sc in range(N_SC)
                ]
                return (gt8_s, xt8_s), dma_th, trans_th, kq_th

            def scores_stationary(gt8_s, j, qt):
                if USE_SWI:
                    return gt8_s[:, j, qt, :]
                return gt8_s[:, 2 * j : 2 * j + 2, qt * P : (qt + 1) * P]

            SC_MODE = SWI if USE_SWI else DR

            def emit_scores_half(gt8_s, xt8_s, qt, h, e2_t, rs_t):
                """One half-row pass for one q-tile: j-major into a [P,2,512]
                two-bank PSUM tile, one 1024-wide exp (fp8 out, offset
                -C_OFF), row-sum via accum_out into rs_t[:, h]."""
                par = qt % 2
                sp = sc_psum.tile([P, 2, NCH], F32, tag="sc")
                for j in range(2):
                    for i in range(2):
                        kc = 2 * h + i
                        nc.tensor.matmul(
                            sp[:, i, :],
                            scores_stationary(gt8_s, j, qt),
                            xt8_s[:, 2 * j : 2 * j + 2, kc * NCH : (kc + 1) * NCH],
                            start=(j == 0),
                            stop=(j == 1),
                            perf_mode=SC_MODE,
                        )
                nc.scalar.activation(
                    e2_t[:, par, h * 2 * NCH : (h + 1) * 2 * NCH],
                    sp[:].rearrange("p a b -> p (a b)"),
                    mybir.ActivationFunctionType.Exp,
                    scale=SCALE / KM,
                    bias=negc_t[:],
                    accum_out=rs_t[:, h : h + 1],
                )

            def emit_r(qt, rs_t, r2_t):
                """r = KR / (rowsumA + rowsumB), broadcast into the qt%2 lane
                of the pair's (interleaved) stationary tile."""
                par = qt % 2
                rtot = soft_pool.tile([P, 1], F32, tag="rtot")
                nc.vector.reduce_sum(rtot[:], rs_t[:], axis=mybir.AxisListType.X)
                rtot_s = soft_pool.tile([P, 1], F32, tag="rtots")
                nc.vector.tensor_scalar_mul(rtot_s[:], rtot[:], 1.0 / KR)
                rrec = soft_pool.tile([P, 1], F32, tag="rrec")
                nc.vector.reciprocal(rrec[:], rtot_s[:])
                if USE_SWI:
                    dst = r2_t[:, par::2]
                else:
                    dst = r2_t[:, par, :]
                nc.vector.tensor_copy(dst, rrec[:, 0:1].broadcast_to([P, P]))

            def colsum_stationary(r2_t):
                if USE_SWI:
                    return r2_t[:, :]
                return r2_t[:, 0:2, :]

            def emit_colsum_pair(w_ps, e2_t, r2_t, pair, kcs, w_off):
                for kc in kcs:
                    nc.tensor.matmul(
                        w_ps[:, kc - w_off, :],
                        colsum_stationary(r2_t),
                        e2_t[:, 0:2, kc * NCH : (kc + 1) * NCH],
                        start=(pair == 0),
                        stop=(pair == N_PAIR - 1),
                        perf_mode=SC_MODE,
                        skip_group_check=True,
                    )

            def phase_A(gt8_s, xt8_s, e2_list, rs_list, extras):
                """Key chunks {0,1} for all 16 q-tiles."""
                for qt in range(N_ST):
                    if qt % 2 == 0:
                        e2_t = e_pool.tile([P, 2, S], F8, tag="e2")
                        e2_list.append(e2_t)
                    rs_t = rs_pool.tile([P, 2], F32, tag="rs")
                    rs_list.append(rs_t)
                    emit_scores_half(gt8_s, xt8_s, qt, 0, e2_list[qt // 2], rs_t)
                    for th in extras[qt]:
                        th()

            def phase_B(gt8_s, xt8_s, e2_list, rs_list, extras):
                """Key chunks {2,3}, r pipeline, inline colsum kc{0,1}
                (2-pair lag, lazy w tile), deferred kc{2,3} sweep thunks."""
                w_holder = {}

                def get_wps():
                    if "a" not in w_holder:
                        w_ps_a = w_psum.tile([P, 2, NCH], F32, tag="w")
                        w_holder["a"] = w_ps_a
                    return w_holder["a"]

                r2_list = []
                pending = []
                for qt in range(N_ST):
                    pair = qt // 2
                    if qt % 2 == 0:
                        if USE_SWI:
                            r2_t = r2_pool.tile([P, 2 * P], F8, tag="r2")
                        else:
                            r2_t = r2_pool.tile([P, 2, P], F8, tag="r2")
                        r2_list.append(r2_t)
                    emit_scores_half(gt8_s, xt8_s, qt, 1, e2_list[pair], rs_list[qt])
                    emit_r(qt, rs_list[qt], r2_list[pair])
                    if qt % 2 == 1:
                        pending.append(pair)
                        if len(pending) > 2:
                            p = pending.pop(0)
                            emit_colsum_pair(
                                get_wps(), e2_list[p], r2_list[p], p, (0, 1), 0
                            )
                    for th in extras[qt]:
                        th()
                for p in pending:
                    emit_colsum_pair(get_wps(), e2_list[p], r2_list[p], p, (0, 1), 0)

                sweep_holder = {}

                def make_sweep_pair(pair):
                    def th():
                        if "b" not in sweep_holder:
                            w_ps_b = w_psum.tile([P, 2, NCH], F32, tag="w")
                            sweep_holder["b"] = w_ps_b
                        emit_colsum_pair(
                            sweep_holder["b"],
                            e2_list[pair],
                            r2_list[pair],
                            pair,
                            (2, 3),
                            2,
                        )

                    return th

                sweep_th = [make_sweep_pair(p) for p in range(N_PAIR)]
                return w_holder["a"], sweep_holder, sweep_th

            def final_thunks(b, w_ps_a, sweep_holder, sweep_th, xnat_s, y_in_w=False):
                """Colsum sweep kc{2,3} + w-phase: out = (w @ X) @ W_v; the
                KR pre-scale is folded into the final o_sb copy.  y_in_w puts
                the y accumulator in the (by then free) w PSUM pool so both
                gp buffers rotate the row->column transposes — without it the
                exposed tail chain serializes at ~560ns/step through one gp
                buffer.  (Only legal for the LAST batch: it adds a w-pool
                allocation.)"""
                w_sb = wvec_pool.tile([1, S], BF16, tag="wsb")
                y_holder = {}

                def get_yps():
                    if "mm" not in y_holder:
                        if y_in_w:
                            y_ps_w = w_psum.tile([P, 2, NCH], F32, tag="w")
                            y_holder["mm"] = y_ps_w[0:1, 0, :]
                        else:
                            y_ps_g = gp_psum.tile([P, NCH], F32, tag="gp")
                            y_holder["mm"] = y_ps_g[0:1, :]
                        y_holder["row"] = y_holder["mm"]
                    return y_holder
                wt_pads = {}
                yt_pads = {}
                thunks = []

                def make_wcopy(kc):
                    def th():
                        src = w_ps_a if kc < 2 else sweep_holder["b"]
                        nc.vector.tensor_copy(
                            w_sb[:, kc * NCH : (kc + 1) * NCH],
                            src[0:1, kc % 2, :],
                        )

                    return th

                def row_to_bcast_cols(src_row, pads, key, tag):
                    """[1,128] SBUF row chunk -> K=1 matmul -> [128,1] PSUM
                    column -> [128,1] SBUF pad.  M=1 matmuls issue ~25%
                    slower than M=128, but the 65ns column copy (vs 290ns
                    broadcast) wins on the exposed tail chain."""
                    tp = gp_psum.tile([P, 1], F32, tag="gp")
                    nc.tensor.matmul(
                        tp[:], src_row, one_t[0:1, 0:1], start=True, stop=True
                    )
                    pad = wvec_pool.tile([P, 1], BF16, tag=tag)
                    nc.vector.tensor_copy(pad[:], tp[:, 0:1])
                    pads[key] = pad

                def make_wtrans(kt):
                    def th():
                        row_to_bcast_cols(
                            w_sb[0:1, kt * P : (kt + 1) * P],
                            wt_pads, kt, f"wtp{kt % 4}",
                        )

                    return th

                def make_ymm(st):
                    def th():
                        nc.tensor.matmul(
                            get_yps()["mm"],
                            wt_pads[st][:],
                            xnat_s[:, st, :],
                            start=(st == 0),
                            stop=(st == N_ST - 1),
                            skip_group_check=True,
                        )

                    return th

                def epilogue_th():
                    y_sb = wvec_pool.tile([1, NCH], BF16, tag="ysb")
                    nc.vector.tensor_copy(y_sb[:], get_yps()["row"])
                    o_ps = gp_psum.tile([P, NCH], F32, tag="gp")
                    for c in range(N_DT):
                        row_to_bcast_cols(
                            y_sb[0:1, c * P : (c + 1) * P], yt_pads, c, f"ytp{c}"
                        )
                    for c in range(N_DT):
                        nc.tensor.matmul(
                            o_ps[0:1, :],
                            yt_pads[c][:],
                            wv_s[:, c, :],
                            start=(c == 0),
                            stop=(c == N_DT - 1),
                            skip_group_check=True,
                        )
                    o_sb = wvec_pool.tile([1, NCH], F32, tag="osb")
                    nc.vector.tensor_scalar_mul(o_sb[:], o_ps[0:1, :], 1.0 / KR)
                    nc.sync.dma_start(out=out_ext[b : b + 1, :], in_=o_sb[:])

                thunks.append(make_wcopy(0))
                thunks.append(make_wcopy(1))
                thunks.extend(sweep_th)
                thunks.append(make_wcopy(2))
                thunks.append(make_wcopy(3))
                for kt in range(N_ST):
                    thunks.append(make_wtrans(kt))
                    if kt >= 3:
                        thunks.append(make_ymm(kt - 3))
                for st in range(N_ST - 3, N_ST):
                    thunks.append(make_ymm(st))
                thunks.append(epilogue_th)
                return thunks

            def spread(thunks, n_slots):
                slots = [[] for _ in range(n_slots)]
                k = len(thunks)
                for i, th in enumerate(thunks):
                    slots[min(i * n_slots // k, n_slots - 1)].append(th)
                return slots

            # ------------------------- emission ------------------------------

            # FILL: s-tile 0 transpose (f32 path), s-tiles 1-3 transposes,
            # M prework, G s-chunk 0 — just enough for phase A0's q-tile 0.
            h0, dma0, trans0, kq0 = proj_thunks(
                0, xnat0_s, x0_loaded, unit_ctx=["fill", "fill", "a0", "a0"]
            )
            g0, xt0 = h0

            def first_tile_trans_f32():
                tp = gp_psum.tile([P, N_DT * P], F32, tag="gp")
                for dt_i in range(N_DT):
                    nc.tensor.matmul(
                        tp[:, dt_i * P : (dt_i + 1) * P],
                        xf0[:, dt_i * P : (dt_i + 1) * P],
                        ident_f[:],
                        start=True,
                        stop=True,
                        skip_group_check=True,
                    )
                nc.vector.tensor_copy(
                    xt0[:, :, 0:P],
                    tp[:].rearrange("p (t c) -> p t c", t=N_DT),
                )

            first_tile_trans_f32()
            pre_th = m_prework_thunks()
            # interleave prework (ACT copies) with c0 transposes (DVE copies);
            # a dummy matmul after each early unit keeps the copy-bound fill
            # phase's PE stream gapless so the clock warms by ~4.5us
            fill_stream = []
            fill_stream.extend(pre_th[:4])  # wkT units
            fill_stream.extend(trans0[0][1:])  # s-tiles 1-3
            fill_stream.extend(pre_th[4:8])  # wqT units
            fill_stream.extend(trans0[1])  # s-tiles 4-7 (phase A needs kc1)
            fill_stream.extend(pre_th[8:])  # M groups
            fill_stream.extend(kq0[0])  # G s-chunk 0
            for th in fill_stream:
                th()

            # batch 1 proj thunks (woven into B0; all DMAs already queued)
            h1, dma1, trans1, kq1 = proj_thunks(1, xnat1_s, [True] * N_SC)
            g1, xt1 = h1

            # --- phase A0: extras = batch0's remaining transposes/G.
            # Deadlines: G sc1 before qt4, sc2 before qt8, sc3 before qt12;
            # trans c2/c3 before phase B0.
            slots_a0 = [[] for _ in range(N_ST)]
            slots_a0[0] = [kq0[1][0], kq0[1][1]]
            slots_a0[1] = [kq0[1][2], kq0[1][3]]
            slots_a0[2] = [trans0[2][0], trans0[2][1]]
            slots_a0[3] = [trans0[2][2], trans0[2][3]]
            slots_a0[4] = [kq0[2][0], kq0[2][1]]
            slots_a0[5] = [kq0[2][2], kq0[2][3]]
            slots_a0[6] = [trans0[3][0], trans0[3][1]]
            slots_a0[7] = [trans0[3][2], trans0[3][3]]
            slots_a0[8] = [kq0[3][0], kq0[3][1]]
            slots_a0[9] = [kq0[3][2], kq0[3][3]]

            e2_0, rs_0 = [], []
            phase_A(g0, xt0, e2_0, rs_0, slots_a0)

            # --- phase B0: extras = batch1's projection.
            proj1_flat = []
            for sc in range(N_SC):
                proj1_flat.extend(trans1[sc])
                proj1_flat.extend(kq1[sc])
            wa0, swh0, swth0 = phase_B(
                g0, xt0, e2_0, rs_0, spread(proj1_flat, N_ST)
            )

            # --- phase A1: extras = batch0's colsum sweep + w-phase.  The
            # sweep matmuls go ONE PAIR PER SLOT — a solid block would park
            # 16 matmuls ahead of the scores stream in the in-order PE queue
            # and starve ACT for ~3.5us.
            fin0 = final_thunks(0, wa0, swh0, swth0, xnat0_s)
            slots_a1 = [[] for _ in range(N_ST)]
            slots_a1[0] = fin0[0:3]  # wcopy0, wcopy1, sweep pair0
            for p in range(1, N_PAIR):
                slots_a1[p] = [fin0[2 + p]]  # sweep pair p
            slots_a1[8].extend(fin0[10:12])  # wcopy2, wcopy3
            rest = fin0[12:]
            k = len(rest)
            for i, th in enumerate(rest):
                slots_a1[8 + min(i * 8 // k, 7)].append(th)
            e2_1, rs_1 = [], []
            phase_A(g1, xt1, e2_1, rs_1, slots_a1)

            # --- phase B1: no extras.
            wa1, swh1, swth1 = phase_B(
                g1, xt1, e2_1, rs_1, [[] for _ in range(N_ST)]
            )

            # --- tail: batch1's sweep + w-phase (y in the free w banks).
            for th in final_thunks(1, wa1, swh1, swth1, xnat1_s, y_in_w=True):
                th()

    nc.compile()
    return nc


_NC_CACHE = None


def _get_nc():
    global _NC_CACHE
    if _NC_CACHE is None:
        _NC_CACHE = build_nc()
    return _NC_CACHE


def make_in_maps(inputs, W_q, W_k, W_v):
    inputs = np.ascontiguousarray(np.asarray(inputs, dtype=np.float32))
    W_q = np.ascontiguousarray(np.asarray(W_q, dtype=np.float32))
    W_k = np.ascontiguousarray(np.asarray(W_k, dtype=np.float32))
    W_v = np.ascontiguousarray(np.asarray(W_v, dtype=np.float32))
    return [
        {
            "inputs": inputs[i * B_PER_CORE : (i + 1) * B_PER_CORE],
            "W_q": W_q,
            "W_k": W_k,
            "W_v": W_v,
        }
        for i in range(N_CORES)
    ]


def kernel(**inputs) -> np.ndarray:
    nc = _get_nc()
    in_maps = make_in_maps(
        inputs["inputs"], inputs["W_q"], inputs["W_k"], inputs["W_v"]
    )
    res = run_bass_kernel_spmd(nc, in_maps, core_ids=list(range(N_CORES)))
    return np.concatenate(
        [res.results[i]["out"] for i in range(N_CORES)], axis=0
    ).astype(np.float32)
